# revision 11
# baseline (speedup 1.0000x reference)
"""Channel-attention (mean + top-4 sum -> shared MLP -> sigmoid gate -> scale)
distributed over 8 TRN2 NeuronCores.

Layout: (b, c) on the 128 SBUF partitions, spatial on the free axis.
Sharding: the D spatial axis is split 8 ways (one contiguous chunk per core).
Per core: stream the 64 MiB local shard once, computing the per-(b,c) running
sum on ScalarE (activation accum_out) and the per-tile top-8 on VectorE
(InstMax) in the same pass; AllGather the tiny [128, 9] per-core stats; merge
(exact top-4 = top-4 of the 8 gathered descending top-8 lists); run the tiny
MLP on TensorE with block-diagonal weights (both batches in one matmul, both
branch stats as the 2-wide moving operand); then stream the shard a second
time multiplying by the per-partition sigmoid gate.
"""

import os
import sys

import numpy as np


def _ensure_imports():
    try:
        import concourse.bass  # noqa: F401
        return
    except ImportError:
        pass
    for p in ("/root/.axon_site/_ro/trn_rl_repo", "/opt/trn_rl_repo"):
        if os.path.isdir(p) and p not in sys.path:
            sys.path.append(p)
    import concourse.bass  # noqa: F401


_ensure_imports()

from concourse import bacc, mybir, tile  # noqa: E402
from concourse.bass_utils import run_bass_kernel_spmd  # noqa: E402

B, C, D, H, W = 2, 64, 64, 128, 128
NCORES = 8
P = B * C                # 128 partitions = (b, c)
DSH = D // NCORES        # 8 D-planes per core
F = DSH * H * W          # 131072 free elements per partition per core
FT = 8192                # free-dim tile size
NT = F // FT             # 16 tiles per pass
TOPK = 4
NCACHE = 6               # leading tiles kept resident in SBUF as bf16
F32 = mybir.dt.float32
BF16 = mybir.dt.bfloat16

_CACHE = {}


def _build():
    nc = bacc.Bacc(
        "TRN2", target_bir_lowering=False, debug=False, num_devices=NCORES
    )
    x_in = nc.declare_dram_parameter("x", [P, F], F32, isOutput=False)
    w1_in = nc.declare_dram_parameter("W1", [C // 2, C], F32, isOutput=False)
    b1_in = nc.declare_dram_parameter("b1", [1, C // 2], F32, isOutput=False)
    w2_in = nc.declare_dram_parameter("W2", [C, C // 2], F32, isOutput=False)
    b2_in = nc.declare_dram_parameter("b2", [1, C], F32, isOutput=False)
    ident_in = nc.declare_dram_parameter("ident", [P, P], F32, isOutput=False)
    out_x = nc.declare_dram_parameter("out", [P, F], F32, isOutput=True)
    gate_out = nc.declare_dram_parameter("gate", [P, 1], F32, isOutput=True)

    cc_in = nc.dram_tensor("cc_in", [9, P], F32)
    cc_out = nc.dram_tensor("cc_out", [NCORES * 9, P], F32, addr_space="Shared")

    AFT = mybir.ActivationFunctionType
    AX = mybir.AxisListType
    rg = [list(range(NCORES))]
    HC = C // 2

    with tile.TileContext(nc) as tc:
        with (
            tc.tile_pool(name="big", bufs=3) as big,
            tc.tile_pool(name="cache", bufs=1) as cache,
            tc.tile_pool(name="small", bufs=1) as small,
            tc.tile_pool(name="ps", bufs=1, space="PSUM") as ps,
        ):
            # Block-diagonal MLP weights: both batches share the MLP, so one
            # [128,64] stationary computes fc1 for b=0 and b=1 at once.
            w1bd = small.tile([P, C], F32)       # lhsT: [k=bc, m=b*32+j]
            w2bd = small.tile([C, P], F32)       # lhsT: [k=b*32+j, m=bc]
            b1bd = small.tile([C, 1], F32)
            b2x2 = small.tile([P, 1], F32)       # 2*b2 (b2 appears in both fc branches)
            ident = small.tile([P, P], F32)
            nc.gpsimd.dma_start(out=ident[:], in_=ident_in[:])
            nc.vector.memset(w1bd[:], 0.0)
            nc.vector.memset(w2bd[:], 0.0)
            for b in range(B):
                nc.gpsimd.dma_start(
                    out=w1bd[b * C:(b + 1) * C, b * HC:(b + 1) * HC],
                    in_=w1_in[:].rearrange("j c -> c j"),
                )
                nc.gpsimd.dma_start(
                    out=w2bd[b * HC:(b + 1) * HC, b * C:(b + 1) * C],
                    in_=w2_in[:].rearrange("c j -> j c"),
                )
                nc.gpsimd.dma_start(
                    out=b1bd[b * HC:(b + 1) * HC, :],
                    in_=b1_in[:].rearrange("a j -> j a"),
                )
                nc.gpsimd.dma_start(
                    out=b2x2[b * C:(b + 1) * C, :],
                    in_=b2_in[:].rearrange("a c -> c a"),
                )
            nc.scalar.mul(b2x2[:], b2x2[:], 2.0)

            # Pass 1: stream the shard; ScalarE accumulates the per-tile sum
            # while VectorE extracts the per-tile top-8. The first NCACHE
            # tiles stay resident in SBUF as bf16 (cast fused into the
            # ScalarE sum-copy) so pass 2 can skip their HBM re-read.
            sum_slots = small.tile([P, NT], F32)
            top8_slots = small.tile([P, NT * 8], F32)
            cached = [
                cache.tile([P, FT], BF16, tag=f"cache{i}", name=f"cache{i}")
                for i in range(NCACHE)
            ]
            for i in range(NT):
                xt = big.tile([P, FT], F32, tag="xt")
                nc.sync.dma_start(out=xt[:], in_=x_in[:, i * FT:(i + 1) * FT])
                acc_dst = cached[i][:] if i < NCACHE else xt[:]
                nc.scalar.activation(
                    acc_dst, xt[:], AFT.Copy, accum_out=sum_slots[:, i:i + 1]
                )
                nc.vector.max(out=top8_slots[:, i * 8:(i + 1) * 8], in_=xt[:])

            # Local merge -> [top8 | sum] = [128, 9]
            stats9 = small.tile([P, 9], F32)
            nc.vector.max(out=stats9[:, 0:8], in_=top8_slots[:])
            nc.vector.reduce_sum(out=stats9[:, 8:9], in_=sum_slots[:], axis=AX.X)

            # Cross-core merge: transpose the stats to [9, 128] on TensorE
            # (identity matmul) so the collective bounce DMAs are
            # partition-contiguous, AllGather, transpose back. The stats DMAs
            # ride the (idle until gate) GpSimd queue so they are not stuck
            # behind prefetch loads in the Sync HWDGE FIFO.
            t9 = ps.tile([9, P], F32)
            nc.tensor.matmul(t9[:], stats9[:], ident[:], start=True, stop=True)
            s9t = small.tile([9, P], F32)
            nc.scalar.copy(s9t[:], t9[:])
            nc.gpsimd.dma_start(out=cc_in[:], in_=s9t[:])
            nc.gpsimd.collective_compute(
                "AllGather",
                mybir.AluOpType.bypass,
                replica_groups=rg,
                ins=[cc_in[:].opt()],
                outs=[cc_out[:].opt()],
            )
            gathT = small.tile([NCORES * 9, P], F32)
            nc.gpsimd.dma_start(out=gathT[:], in_=cc_out[:])
            t72 = ps.tile([P, NCORES * 9], F32)
            nc.tensor.matmul(
                t72[:], gathT[:], ident[0:NCORES * 9, 0:NCORES * 9],
                start=True, stop=True,
            )
            gath = small.tile([P, NCORES * 9], F32)
            nc.scalar.copy(gath[:], t72[:])

            # Global merge: exact top-4 = first 4 of top-8 of the gathered
            # per-core descending top-8 lists; mean from the summed sums.
            g_top8 = small.tile([P, 8], F32)
            gv = gath[:].rearrange("p (r e) -> p r e", e=9)
            nc.vector.max(out=g_top8[:], in_=gv[:, :, 0:8])
            stats2 = small.tile([P, 2], F32)
            gs = gath[:].rearrange("p (r e) -> p e r", e=9)
            nc.vector.reduce_sum(out=stats2[:, 0:1], in_=gs[:, 8:9, :], axis=AX.X)
            nc.scalar.mul(stats2[:, 0:1], stats2[:, 0:1], 1.0 / (D * H * W))
            nc.vector.reduce_sum(out=stats2[:, 1:2], in_=g_top8[:, 0:4], axis=AX.X)

            # MLP: h = relu(W1 @ stats + b1); s = W2 @ h; gate = sigmoid(s0 + s1 + 2*b2)
            h_ps = ps.tile([C, B], F32)
            nc.tensor.matmul(h_ps[:], w1bd[:], stats2[:], start=True, stop=True)
            h_sb = small.tile([C, B], F32)
            nc.scalar.activation(h_sb[:], h_ps[:], AFT.Relu, bias=b1bd[:], scale=1.0)
            s_ps = ps.tile([P, B], F32)
            nc.tensor.matmul(s_ps[:], w2bd[:], h_sb[:], start=True, stop=True)
            ssum = small.tile([P, 1], F32)
            nc.vector.reduce_sum(out=ssum[:], in_=s_ps[:], axis=AX.X)
            gate_sb = small.tile([P, 1], F32)
            nc.scalar.activation(gate_sb[:], ssum[:], AFT.Sigmoid, bias=b2x2[:], scale=1.0)

            # Pass 2: scale by the per-partition gate. Cached tiles go first
            # (they only need the gate): in-place bf16 multiply on DVE (4x
            # mode), store with cast-on-DMA via SWDGE -- no staging tile, no
            # slot contention, and their stores flow while the streamed
            # pipeline refills. Streamed tiles alternate the multiply between
            # DVE and ScalarE so an in-order stall on one engine does not
            # block the other's tiles. Loads ride the Sync HWDGE queue,
            # stores the Scalar HWDGE queue.
            for i in range(NCACHE):
                nc.vector.tensor_scalar_mul(cached[i][:], cached[i][:], gate_sb[:])
                nc.gpsimd.dma_start(out=out_x[:, i * FT:(i + 1) * FT], in_=cached[i][:])
            for k, i in enumerate(range(NCACHE, NT)):
                yt = big.tile([P, FT], F32, tag="xt")
                nc.sync.dma_start(out=yt[:], in_=x_in[:, i * FT:(i + 1) * FT])
                if k % 2 == 0:
                    nc.vector.tensor_scalar_mul(yt[:], yt[:], gate_sb[:])
                else:
                    nc.scalar.mul(yt[:], yt[:], gate_sb[:])
                nc.scalar.dma_start(out=out_x[:, i * FT:(i + 1) * FT], in_=yt[:])
            nc.gpsimd.dma_start(out=gate_out[:], in_=gate_sb[:])

    nc.compile()
    return nc


def _get_nc():
    if "nc" not in _CACHE:
        _CACHE["nc"] = _build()
    return _CACHE["nc"]


def _make_in_maps(x, W1, b1, W2, b2):
    x = np.ascontiguousarray(x, dtype=np.float32)
    w1 = np.ascontiguousarray(W1, dtype=np.float32)
    b1r = np.ascontiguousarray(b1, dtype=np.float32).reshape(1, HC_CONST)
    w2 = np.ascontiguousarray(W2, dtype=np.float32)
    b2r = np.ascontiguousarray(b2, dtype=np.float32).reshape(1, C)
    ident = np.eye(P, dtype=np.float32)
    in_maps = []
    for i in range(NCORES):
        shard = np.ascontiguousarray(
            x[:, :, i * DSH:(i + 1) * DSH]
        ).reshape(P, F)
        in_maps.append(
            {"x": shard, "W1": w1, "b1": b1r, "W2": w2, "b2": b2r,
             "ident": ident}
        )
    return in_maps


HC_CONST = C // 2


def run_device(x, W1, b1, W2, b2, trace=False, **kwargs):
    """Run the SPMD kernel; returns (BassKernelResults, assembled outputs)."""
    nc = _get_nc()
    in_maps = _make_in_maps(x, W1, b1, W2, b2)
    res = run_bass_kernel_spmd(
        nc, in_maps, core_ids=list(range(NCORES)), trace=trace, **kwargs
    )
    scaled = np.empty((B, C, D, H, W), dtype=np.float32)
    for i in range(NCORES):
        scaled[:, :, i * DSH:(i + 1) * DSH] = (
            res.results[i]["out"].reshape(B, C, DSH, H, W)
        )
    gate = res.results[0]["gate"].reshape(B, C)
    return res, (scaled, gate)


def kernel(x, W1, b1, W2, b2):
    _, outs = run_device(x, W1, b1, W2, b2, trace=False)
    return outs
